# revision 18
# baseline (speedup 1.0000x reference)
"""Trainium2 Bass kernel: 2D valid cross-correlation (3x3) + bias on 8192x8192 fp32.

Strategy (v7):
  - Row-shard X across 8 NeuronCores with a 2-row halo handled by host-side
    overlapped slicing (each core gets a 1026x8192 slab; core 7's slab is
    shifted up by 2 rows so all cores run an identical SPMD program).
  - The kernel is memory-bound, so both directions ride bf16: X is rounded to
    bf16 on the host before upload and the output is written as bf16 and
    upcast on the host. That halves HBM traffic to ~34MB/core (~3e-3 rel err,
    well inside the 2e-2 budget). The per-NC DMA aggregate caps at ~370GB/s,
    so the DMA floor is ~92us; the PE (3 dj-passes x 8190 cols x 8 slabs of
    126 rows, bf16 at 1 col/cycle) runs ~85us ending just before the DMA.
  - conv2d(3x3) = 3 PSUM-accumulating matmuls per output tile with banded
    "shift" matrices B_dj[p, o] = w[p - o, dj]; row shifts ride the matmul
    contraction, column shifts dj are free-axis offsets of the rhs.
  - Loads ride the sync HWDGE ring in ~0.5MB column chunks; stores ride SWDGE
    (gpsimd) whose descriptor swizzle spreads 126-partition tiles over all 16
    SDMA engines (HWDGE uses only 14). The final slab's stores split across
    the two HWDGE rings, which are idle by then.
  - A short burst of dummy matmuls on a zeroed scratch tile bridges the ~5us
    preamble so the PE HAM clock gate opens before real work arrives.
  - The 16-row tail slab packs its matmuls 4-wide into PE column groups
    (tile_position via psum base-partition 32t; bands zero-padded to 32/30
    output columns so every psum partition is written). Its four column
    groups are emitted after slabs 3-6, hiding their small evict+store
    chains under later slabs' compute instead of dangling at the end.
"""

import os
import sys

import numpy as np

for _p in (
    "/opt/trn_rl_repo",
    "/root/.axon_site/_ro/trn_rl_repo",
    "/root/.axon_site/_ro/pypackages",
    "/opt/pypackages",
):
    if os.path.isdir(_p) and _p not in sys.path:
        sys.path.append(_p)

import concourse.bacc as bacc
import concourse.mybir as mybir
import concourse.tile as tile
from concourse.bass_utils import run_bass_kernel_spmd

N_CORES = 8
H = W = 8192
KH = KW = 3
OH = H - KH + 1  # 8190
OW = W - KW + 1  # 8190
ROWS_PER_CORE = 1024  # output rows produced per core (core 7: first 2 dropped)
SLAB_IN_ROWS = 1026  # input rows per core slab
SLAB_IN = 128  # input rows per row-slab tile
SLAB_OUT = 126  # output rows per row-slab tile
N_FULL_SLABS = 8  # 8 * 126 = 1008
TAIL_IN = SLAB_IN_ROWS - N_FULL_SLABS * SLAB_OUT  # 18
TAIL_OUT = ROWS_PER_CORE - N_FULL_SLABS * SLAB_OUT  # 16
TAIL_R0 = N_FULL_SLABS * SLAB_OUT  # 1008
TAIL_M = 32  # zero-padded tail band output cols (30 for the last col group)
COL_TILE = 512
GROUP = 4  # col-tiles per dj-outer matmul group (= PSUM banks per group)
N_GROUPS = 4
GROUP_COLS = GROUP * COL_TILE  # 2048 output cols per group
CHUNK_W = GROUP_COLS + KW - 1  # 2050 input cols per chunk (2-col halo)
SUB_W = COL_TILE + KW - 1  # 514: slab-0 group-0 fine-grained sub-chunk
WARMUP_MMS = 2  # dummy matmuls bridging PE start to first-load arrival: real
# matmuls take over cold (~427ns) until the HAM window opens ~4.6us after the
# first matmul; that costs less than padding the stream with more dummies.

BF16 = mybir.dt.bfloat16
F32 = mybir.dt.float32
NP_BF16 = mybir.dt.np(BF16)

_NC = None
LAST_RESULTS = None


def _build():
    nc = bacc.Bacc(
        "TRN2", target_bir_lowering=False, debug=False, num_devices=N_CORES
    )
    xs = nc.dram_tensor("xs", [SLAB_IN_ROWS, W], BF16, kind="ExternalInput")
    bands = nc.dram_tensor("bands", [SLAB_IN, KW, SLAB_OUT], BF16, kind="ExternalInput")
    bandt = nc.dram_tensor("bandt", [TAIL_IN, KW, TAIL_M], BF16, kind="ExternalInput")
    out = nc.dram_tensor("out", [ROWS_PER_CORE, OW], BF16, kind="ExternalOutput")

    with tile.TileContext(nc) as tc:
        with (
            tc.tile_pool(name="const", bufs=1) as cpool,
            tc.tile_pool(name="inp", bufs=4) as ipool,
            tc.tile_pool(name="fine", bufs=1) as fpool,
            tc.tile_pool(name="outp", bufs=2) as opool,
            tc.tile_pool(name="tailp", bufs=2) as tpool,
            tc.tile_pool(name="psum", bufs=2 * GROUP, space="PSUM") as pspool,
        ):
            # PE warm-up: the HAM clock gate keeps the PE at 1.2GHz until it
            # has been busy for a ~3.4us window. Dummy matmuls on a zeroed
            # scratch tile (no DMA dependencies) run during the preamble/load
            # ramp so the real matmuls start warm at 2.4GHz.
            scratch = cpool.tile([SLAB_IN, COL_TILE], BF16, tag="scr")
            nc.vector.memset(scratch[:], 0.0)
            wps = pspool.tile([SLAB_OUT, COL_TILE], F32, tag="ps", name="warm_ps")
            for i in range(WARMUP_MMS):
                nc.tensor.matmul(
                    wps[:, :256],
                    scratch[:, :SLAB_OUT],
                    scratch[:, :256],
                    start=True,
                    stop=True,
                )

            # Bands ride the sync (HWDGE) ring ahead of the first slab chunks:
            # tiny transfers that gate the first matmuls, so they go first.
            bt = cpool.tile([SLAB_IN, KW, SLAB_OUT], BF16, tag="bt")
            nc.sync.dma_start(bt[:], bands.ap())

            # Tail band + input chunks (tiny, 18 rows) load right after slab
            # 0's tiles — the tail's column groups only run after slabs 3-6,
            # and loading them first would delay the first matmul's data.
            btt = cpool.tile([TAIL_IN, KW, TAIL_M], BF16, tag="btt")
            tcks = [
                fpool.tile([SLAB_IN, CHUNK_W], BF16, tag=f"t{g}", name=f"cktail_{g}")
                for g in range(N_GROUPS)
            ]

            def load_tail_inputs():
                nc.sync.dma_start(btt[:], bandt.ap())
                for g in range(N_GROUPS):
                    c0 = g * GROUP_COLS
                    w = min(CHUNK_W, W - c0)
                    nc.sync.dma_start(
                        tcks[g][:TAIL_IN, :w],
                        xs.ap()[TAIL_R0 : TAIL_R0 + TAIL_IN, c0 : c0 + w],
                    )

            def tail_group(g):
                gc0 = g * GROUP_COLS
                ps = pspool.tile([SLAB_OUT, COL_TILE], F32, tag="ps",
                                 name=f"pstail_{g}")
                for dj in range(KW):
                    for t in range(GROUP):
                        n = min(COL_TILE, OW - (gc0 + t * COL_TILE))
                        lo = t * COL_TILE + dj
                        m = TAIL_M if t < GROUP - 1 else SLAB_OUT - 32 * (GROUP - 1)
                        # Four col-tiles run concurrently in PE column groups.
                        # Explicit tile_position: auto-derive rejects psum
                        # base partition 96. The band is zero-padded to m
                        # output cols so all 126 psum partitions get written
                        # and one full-width eviction suffices.
                        nc.tensor.matmul(
                            ps[32 * t : 32 * t + m, :n],
                            btt[:TAIL_IN, dj, :m],
                            tcks[g][:TAIL_IN, lo : lo + n],
                            start=(dj == 0),
                            stop=(dj == KW - 1),
                            tile_position=(0, 32 * t),
                        )
                tot = tpool.tile([SLAB_OUT, COL_TILE], BF16, tag="tot",
                                 name=f"tot{g}")
                nc.vector.tensor_copy(tot[:, :], ps[:, :])
                for t in range(GROUP):
                    c0 = gc0 + t * COL_TILE
                    n = min(COL_TILE, OW - c0)
                    # Mid-stream stores must stay OFF the HWDGE rings: those
                    # sequencers execute in order, so a store blocked on its
                    # eviction would stall every load queued behind it.
                    nc.gpsimd.dma_start(
                        out.ap()[TAIL_R0 : TAIL_R0 + TAIL_OUT, c0 : c0 + n],
                        tot[32 * t : 32 * t + TAIL_OUT, :n],
                    )

            for s in range(N_FULL_SLABS):
                in_rows = SLAB_IN
                out_rows = SLAB_OUT
                r0 = s * SLAB_OUT

                # One input tile per column group so each group's matmuls only
                # depend on their own ~0.5MB chunk. Slab 0 group 0 (the tiles
                # gating the first full-rate matmuls) splits further into four
                # 514-col sub-tiles so compute starts after ~0.13MB.
                cks = []
                fine = None
                for g in range(N_GROUPS):
                    c0 = g * GROUP_COLS
                    w = min(CHUNK_W, W - c0)
                    if s == 0 and g == 0:
                        fine = []
                        for t in range(GROUP):
                            fc0 = t * COL_TILE
                            ck = fpool.tile(
                                [SLAB_IN, SUB_W], BF16, tag=f"f{t}", name=f"fine{t}"
                            )
                            nc.sync.dma_start(
                                ck[:, :], xs.ap()[r0 : r0 + SLAB_IN, fc0 : fc0 + SUB_W]
                            )
                            fine.append(ck)
                        cks.append(None)
                        continue
                    ck = ipool.tile([SLAB_IN, CHUNK_W], BF16, tag=f"ck{g}",
                                    name=f"ck{s}_{g}")
                    nc.sync.dma_start(
                        ck[:in_rows, :w], xs.ap()[r0 : r0 + in_rows, c0 : c0 + w]
                    )
                    cks.append(ck)

                ot = opool.tile([SLAB_OUT, OW], BF16, tag="ot", name=f"ot{s}")

                for g in range(N_GROUPS):
                    gc0 = g * GROUP_COLS
                    pss = [
                        pspool.tile(
                            [SLAB_OUT, COL_TILE], F32, tag="ps", name=f"ps{s}_{g}_{t}"
                        )
                        for t in range(GROUP)
                    ]
                    for dj in range(KW):
                        for t in range(GROUP):
                            n = min(COL_TILE, OW - (gc0 + t * COL_TILE))
                            if s == 0 and g == 0:
                                rhs = fine[t][:in_rows, dj : dj + n]
                            else:
                                lo = t * COL_TILE + dj
                                rhs = cks[g][:in_rows, lo : lo + n]
                            nc.tensor.matmul(
                                pss[t][:out_rows, :n],
                                bt[:in_rows, dj, :out_rows],
                                rhs,
                                start=(dj == 0),
                                stop=(dj == KW - 1),
                            )
                    for t in range(GROUP):
                        c0 = gc0 + t * COL_TILE
                        n = min(COL_TILE, OW - c0)
                        # Cast-copy PSUM->SBUF, alternating DVE / ACT so
                        # neither engine sits on the critical path.
                        if t % 2 == 0:
                            nc.vector.tensor_copy(
                                ot[:out_rows, c0 : c0 + n], pss[t][:out_rows, :n]
                            )
                        else:
                            nc.scalar.copy(
                                ot[:out_rows, c0 : c0 + n], pss[t][:out_rows, :n]
                            )
                    a = gc0
                    b = min(gc0 + GROUP_COLS, OW)
                    # SWDGE stores: the gpsimd descriptor swizzle spreads the
                    # 126 partition lines across all 16 SDMA engines, while
                    # HWDGE puts a 126-partition store on only 14 of them.
                    # The last slab's stores split across the two HWDGE rings
                    # instead: loads are done by then, and HWDGE issue and
                    # completion are faster, shortening the final drain.
                    if s == N_FULL_SLABS - 1:
                        if g == N_GROUPS - 1:
                            # Split the very last store across both HWDGE
                            # rings so the final drain halves.
                            m = (a + b) // 2
                            nc.sync.dma_start(
                                out.ap()[r0 : r0 + out_rows, a:m], ot[:out_rows, a:m]
                            )
                            nc.scalar.dma_start(
                                out.ap()[r0 : r0 + out_rows, m:b], ot[:out_rows, m:b]
                            )
                        else:
                            eng = nc.sync if g % 2 == 0 else nc.scalar
                            eng.dma_start(
                                out.ap()[r0 : r0 + out_rows, a:b], ot[:out_rows, a:b]
                            )
                    else:
                        nc.gpsimd.dma_start(
                            out.ap()[r0 : r0 + out_rows, a:b], ot[:out_rows, a:b]
                        )
                if s == 0:
                    load_tail_inputs()
                # Hide the tail's evict+store chains under slabs 4-7.
                if 3 <= s <= 6:
                    tail_group(s - 3)

    nc.compile()
    return nc


def kernel(X, weight, bias):
    global _NC, LAST_RESULTS
    X = np.asarray(X, dtype=np.float32)
    weight = np.asarray(weight, dtype=np.float32)
    bias = np.asarray(bias, dtype=np.float32).reshape(-1)

    if _NC is None:
        _NC = _build()
    nc = _NC

    xbf = np.ascontiguousarray(X.astype(NP_BF16))

    # Banded shift matrices: bands[p, dj, o] = w[p - o, dj] for 0 <= p-o < 3.
    bands = np.zeros((SLAB_IN, KW, SLAB_OUT), dtype=np.float32)
    o = np.arange(SLAB_OUT)
    for di in range(KH):
        for dj in range(KW):
            bands[o + di, dj, o] = weight[di, dj]
    bands = bands.astype(NP_BF16)

    # Tail bands: 16 real output cols zero-padded to TAIL_M so the packed
    # matmuls write every psum partition they cover.
    bandt = np.zeros((TAIL_IN, KW, TAIL_M), dtype=np.float32)
    ot_ = np.arange(TAIL_OUT)
    for di in range(KH):
        for dj in range(KW):
            bandt[ot_ + di, dj, ot_] = weight[di, dj]
    bandt = bandt.astype(NP_BF16)

    starts = [min(i * ROWS_PER_CORE, H - SLAB_IN_ROWS) for i in range(N_CORES)]
    in_maps = [
        {
            "xs": np.ascontiguousarray(xbf[s0 : s0 + SLAB_IN_ROWS]),
            "bands": bands,
            "bandt": bandt,
        }
        for s0 in starts
    ]

    # The shared device occasionally returns corrupted results after an NRT
    # wedge (observed once across dozens of runs: rel err jumped ~12 orders of
    # magnitude on an unchanged binary). A handful of sampled rows checked
    # against a host conv (~2M flops) catches that reliably — the legit bf16
    # error is ~1e-2 absolute while corruption shows up as O(10+) — so retry
    # the device run when the spot check fails.
    for attempt in range(3):
        res = run_bass_kernel_spmd(nc, in_maps, core_ids=list(range(N_CORES)))
        LAST_RESULTS = res

        full = np.empty((OH, OW), dtype=np.float32)
        for i in range(N_CORES - 1):
            full[i * ROWS_PER_CORE : (i + 1) * ROWS_PER_CORE] = np.asarray(
                res.results[i]["out"], dtype=np.float32
            )
        # Core 7's slab starts at row 7166, so its first 2 output rows
        # duplicate core 6's last 2; keep rows 2.. (= conv rows 7168..8189).
        full[(N_CORES - 1) * ROWS_PER_CORE :] = np.asarray(
            res.results[N_CORES - 1]["out"], dtype=np.float32
        )[ROWS_PER_CORE - (OH - (N_CORES - 1) * ROWS_PER_CORE) :]
        if bias[0] != 0.0:
            full += bias[0]
        if _spot_check(full, xbf, weight, bias[0]):
            return full
        print(
            f"kernel: device output failed spot check (attempt {attempt + 1}); "
            "retrying",
            file=sys.stderr,
        )
    return full


def _spot_check(full, xbf, w, bias):
    rows = set()
    for i in range(N_CORES):
        base = i * ROWS_PER_CORE
        rows.update((base, base + 513, base + SLAB_OUT * 4, base + 1023))
    rows.add(OH - 1)
    wq = w.astype(NP_BF16).astype(np.float32)
    for r in sorted(rows):
        if r >= OH:
            continue
        xr = xbf[r : r + KH].astype(np.float32)
        ref = np.zeros(OW, dtype=np.float32)
        for di in range(KH):
            for dj in range(KW):
                ref += wq[di, dj] * xr[di, dj : dj + OW]
        ref += bias
        tol = max(0.05 * float(np.abs(ref).max()), 0.05)
        if float(np.abs(full[r] - ref).max()) > tol:
            return False
    return True


# revision 21
# speedup vs baseline: 1.0651x; 1.0651x over previous
"""Trainium2 Bass kernel: 2D valid cross-correlation (3x3) + bias on 8192x8192 fp32.

Strategy (v7):
  - Row-shard X across 8 NeuronCores with a 2-row halo handled by host-side
    overlapped slicing (each core gets a 1026x8192 slab; core 7's slab is
    shifted up by 2 rows so all cores run an identical SPMD program).
  - The kernel is memory-bound, so both directions ride bf16: X is rounded to
    bf16 on the host before upload and the output is written as bf16 and
    upcast on the host. That halves HBM traffic to ~34MB/core (~3e-3 rel err,
    well inside the 2e-2 budget). The per-NC DMA aggregate caps at ~370GB/s,
    so the DMA floor is ~92us; the PE (3 dj-passes x 8190 cols x 8 slabs of
    126 rows, bf16 at 1 col/cycle) runs ~85us ending just before the DMA.
  - conv2d(3x3) = 3 PSUM-accumulating matmuls per output tile with banded
    "shift" matrices B_dj[p, o] = w[p - o, dj]; row shifts ride the matmul
    contraction, column shifts dj are free-axis offsets of the rhs.
  - Loads ride the sync HWDGE ring in ~0.5MB column chunks; stores ride SWDGE
    (gpsimd) whose descriptor swizzle spreads 126-partition tiles over all 16
    SDMA engines (HWDGE uses only 14). The final slab's stores split across
    the two HWDGE rings, which are idle by then.
  - A short burst of dummy matmuls on a zeroed scratch tile bridges the ~5us
    preamble so the PE HAM clock gate opens before real work arrives.
  - The 16-row tail slab packs its matmuls 4-wide into PE column groups
    (tile_position via psum base-partition 32t; bands zero-padded to 32/30
    output columns so every psum partition is written). Its four column
    groups are emitted after slabs 3-6, hiding their small evict+store
    chains under later slabs' compute instead of dangling at the end.
"""

import os
import sys

import numpy as np

for _p in (
    "/opt/trn_rl_repo",
    "/root/.axon_site/_ro/trn_rl_repo",
    "/root/.axon_site/_ro/pypackages",
    "/opt/pypackages",
):
    if os.path.isdir(_p) and _p not in sys.path:
        sys.path.append(_p)

import concourse.bacc as bacc
import concourse.mybir as mybir
import concourse.tile as tile
from concourse.bass_utils import run_bass_kernel_spmd

N_CORES = 8
H = W = 8192
KH = KW = 3
OH = H - KH + 1  # 8190
OW = W - KW + 1  # 8190
ROWS_PER_CORE = 1024  # output rows produced per core (core 7: first 2 dropped)
SLAB_IN_ROWS = 1026  # input rows per core slab
SLAB_IN = 128  # input rows per row-slab tile
SLAB_OUT = 126  # output rows per row-slab tile
N_FULL_SLABS = 8  # 8 * 126 = 1008
TAIL_IN = SLAB_IN_ROWS - N_FULL_SLABS * SLAB_OUT  # 18
TAIL_OUT = ROWS_PER_CORE - N_FULL_SLABS * SLAB_OUT  # 16
TAIL_R0 = N_FULL_SLABS * SLAB_OUT  # 1008
TAIL_M = 32  # zero-padded tail band output cols (30 for the last col group)
COL_TILE = 512
GROUP = 4  # col-tiles per dj-outer matmul group (= PSUM banks per group)
N_GROUPS = 4
GROUP_COLS = GROUP * COL_TILE  # 2048 output cols per group
CHUNK_W = GROUP_COLS + KW - 1  # 2050 input cols per chunk (2-col halo)
SUB_W = COL_TILE + KW - 1  # 514: slab-0 group-0 fine-grained sub-chunk
WARMUP_MMS = 20  # dummy matmuls bridging PE start to first-load arrival (the
# sync sequencer issues loads at ~1us each, so real data lands ~12us in; the
# warmup also opens the HAM clock gate so real matmuls start at 2.4GHz).

BF16 = mybir.dt.bfloat16
F32 = mybir.dt.float32
NP_BF16 = mybir.dt.np(BF16)

_NC = None
LAST_RESULTS = None


def _build():
    nc = bacc.Bacc(
        "TRN2", target_bir_lowering=False, debug=False, num_devices=N_CORES
    )
    xs = nc.dram_tensor("xs", [SLAB_IN_ROWS, W], BF16, kind="ExternalInput")
    bands = nc.dram_tensor("bands", [SLAB_IN, KW, SLAB_OUT], BF16, kind="ExternalInput")
    bandt = nc.dram_tensor("bandt", [TAIL_IN, KW, TAIL_M], BF16, kind="ExternalInput")
    out = nc.dram_tensor("out", [ROWS_PER_CORE, OW], BF16, kind="ExternalOutput")

    with tile.TileContext(nc) as tc:
        with (
            tc.tile_pool(name="const", bufs=1) as cpool,
            tc.tile_pool(name="inp", bufs=4) as ipool,
            tc.tile_pool(name="fine", bufs=1) as fpool,
            tc.tile_pool(name="outp", bufs=2) as opool,
            tc.tile_pool(name="tailp", bufs=2) as tpool,
            tc.tile_pool(name="psum", bufs=2 * GROUP, space="PSUM") as pspool,
        ):
            # PE warm-up: the HAM clock gate keeps the PE at 1.2GHz until it
            # has been busy for a ~3.4us window. Dummy matmuls on a zeroed
            # scratch tile (no DMA dependencies) run during the preamble/load
            # ramp so the real matmuls start warm at 2.4GHz.
            scratch = cpool.tile([SLAB_IN, COL_TILE], BF16, tag="scr")
            nc.vector.memset(scratch[:], 0.0)
            wps = pspool.tile([SLAB_OUT, COL_TILE], F32, tag="ps", name="warm_ps")
            for i in range(WARMUP_MMS):
                nc.tensor.matmul(
                    wps[:, :256],
                    scratch[:, :SLAB_OUT],
                    scratch[:, :256],
                    start=True,
                    stop=True,
                )

            # Bands ride the sync (HWDGE) ring ahead of the first slab chunks:
            # tiny transfers that gate the first matmuls, so they go first.
            bt = cpool.tile([SLAB_IN, KW, SLAB_OUT], BF16, tag="bt")
            nc.sync.dma_start(bt[:], bands.ap())

            # Tail band + input chunks (tiny, 18 rows) load right after slab
            # 0's tiles — the tail's column groups only run after slabs 3-6,
            # and loading them first would delay the first matmul's data.
            btt = cpool.tile([TAIL_IN, KW, TAIL_M], BF16, tag="btt")
            tcks = [
                fpool.tile([SLAB_IN, CHUNK_W], BF16, tag=f"t{g}", name=f"cktail_{g}")
                for g in range(N_GROUPS)
            ]

            def load_tail_inputs():
                nc.sync.dma_start(btt[:], bandt.ap())
                for g in range(N_GROUPS):
                    c0 = g * GROUP_COLS
                    w = min(CHUNK_W, W - c0)
                    nc.sync.dma_start(
                        tcks[g][:TAIL_IN, :w],
                        xs.ap()[TAIL_R0 : TAIL_R0 + TAIL_IN, c0 : c0 + w],
                    )

            def tail_group(g):
                gc0 = g * GROUP_COLS
                ps = pspool.tile([SLAB_OUT, COL_TILE], F32, tag="ps",
                                 name=f"pstail_{g}")
                for dj in range(KW):
                    for t in range(GROUP):
                        n = min(COL_TILE, OW - (gc0 + t * COL_TILE))
                        lo = t * COL_TILE + dj
                        m = TAIL_M if t < GROUP - 1 else SLAB_OUT - 32 * (GROUP - 1)
                        # Four col-tiles run concurrently in PE column groups.
                        # Explicit tile_position: auto-derive rejects psum
                        # base partition 96. The band is zero-padded to m
                        # output cols so all 126 psum partitions get written
                        # and one full-width eviction suffices.
                        nc.tensor.matmul(
                            ps[32 * t : 32 * t + m, :n],
                            btt[:TAIL_IN, dj, :m],
                            tcks[g][:TAIL_IN, lo : lo + n],
                            start=(dj == 0),
                            stop=(dj == KW - 1),
                            tile_position=(0, 32 * t),
                        )
                tot = tpool.tile([SLAB_OUT, COL_TILE], BF16, tag="tot",
                                 name=f"tot{g}")
                nc.vector.tensor_copy(tot[:, :], ps[:, :])
                for t in range(GROUP):
                    c0 = gc0 + t * COL_TILE
                    n = min(COL_TILE, OW - c0)
                    # Mid-stream stores must stay OFF the HWDGE rings: those
                    # sequencers execute in order, so a store blocked on its
                    # eviction would stall every load queued behind it.
                    nc.gpsimd.dma_start(
                        out.ap()[TAIL_R0 : TAIL_R0 + TAIL_OUT, c0 : c0 + n],
                        tot[32 * t : 32 * t + TAIL_OUT, :n],
                    )

            for s in range(N_FULL_SLABS):
                in_rows = SLAB_IN
                out_rows = SLAB_OUT
                r0 = s * SLAB_OUT

                # One input tile per column group so each group's matmuls only
                # depend on their own ~0.5MB chunk (the sync sequencer costs
                # ~1us per dma_start, so fewer/bigger issues beat fine splits).
                cks = []
                for g in range(N_GROUPS):
                    c0 = g * GROUP_COLS
                    w = min(CHUNK_W, W - c0)
                    ck = ipool.tile([SLAB_IN, CHUNK_W], BF16, tag=f"ck{g}",
                                    name=f"ck{s}_{g}")
                    nc.sync.dma_start(
                        ck[:in_rows, :w], xs.ap()[r0 : r0 + in_rows, c0 : c0 + w]
                    )
                    cks.append(ck)

                ot = opool.tile([SLAB_OUT, OW], BF16, tag="ot", name=f"ot{s}")

                for g in range(N_GROUPS):
                    gc0 = g * GROUP_COLS
                    pss = [
                        pspool.tile(
                            [SLAB_OUT, COL_TILE], F32, tag="ps", name=f"ps{s}_{g}_{t}"
                        )
                        for t in range(GROUP)
                    ]
                    for dj in range(KW):
                        for t in range(GROUP):
                            n = min(COL_TILE, OW - (gc0 + t * COL_TILE))
                            lo = t * COL_TILE + dj
                            rhs = cks[g][:in_rows, lo : lo + n]
                            nc.tensor.matmul(
                                pss[t][:out_rows, :n],
                                bt[:in_rows, dj, :out_rows],
                                rhs,
                                start=(dj == 0),
                                stop=(dj == KW - 1),
                            )
                    for t in range(GROUP):
                        c0 = gc0 + t * COL_TILE
                        n = min(COL_TILE, OW - c0)
                        # Cast-copy PSUM->SBUF, alternating DVE / ACT so
                        # neither engine sits on the critical path.
                        if t % 2 == 0:
                            nc.vector.tensor_copy(
                                ot[:out_rows, c0 : c0 + n], pss[t][:out_rows, :n]
                            )
                        else:
                            nc.scalar.copy(
                                ot[:out_rows, c0 : c0 + n], pss[t][:out_rows, :n]
                            )
                    a = gc0
                    b = min(gc0 + GROUP_COLS, OW)
                    # SWDGE stores: the gpsimd descriptor swizzle spreads the
                    # 126 partition lines across all 16 SDMA engines, while
                    # HWDGE puts a 126-partition store on only 14 of them.
                    # The last slab's stores split across the two HWDGE rings
                    # instead: loads are done by then, and HWDGE issue and
                    # completion are faster, shortening the final drain.
                    if s == N_FULL_SLABS - 1:
                        if g == N_GROUPS - 1:
                            # Split the very last store across both HWDGE
                            # rings so the final drain halves.
                            m = (a + b) // 2
                            nc.sync.dma_start(
                                out.ap()[r0 : r0 + out_rows, a:m], ot[:out_rows, a:m]
                            )
                            nc.scalar.dma_start(
                                out.ap()[r0 : r0 + out_rows, m:b], ot[:out_rows, m:b]
                            )
                        else:
                            eng = nc.sync if g % 2 == 0 else nc.scalar
                            eng.dma_start(
                                out.ap()[r0 : r0 + out_rows, a:b], ot[:out_rows, a:b]
                            )
                    else:
                        nc.gpsimd.dma_start(
                            out.ap()[r0 : r0 + out_rows, a:b], ot[:out_rows, a:b]
                        )
                if s == 0:
                    load_tail_inputs()
                # Hide the tail's evict+store chains under slabs 4-7.
                if 3 <= s <= 6:
                    tail_group(s - 3)

    nc.compile()
    return nc


def kernel(X, weight, bias):
    global _NC, LAST_RESULTS
    X = np.asarray(X, dtype=np.float32)
    weight = np.asarray(weight, dtype=np.float32)
    bias = np.asarray(bias, dtype=np.float32).reshape(-1)

    if _NC is None:
        _NC = _build()
    nc = _NC

    xbf = np.ascontiguousarray(X.astype(NP_BF16))

    # Banded shift matrices: bands[p, dj, o] = w[p - o, dj] for 0 <= p-o < 3.
    bands = np.zeros((SLAB_IN, KW, SLAB_OUT), dtype=np.float32)
    o = np.arange(SLAB_OUT)
    for di in range(KH):
        for dj in range(KW):
            bands[o + di, dj, o] = weight[di, dj]
    bands = bands.astype(NP_BF16)

    # Tail bands: 16 real output cols zero-padded to TAIL_M so the packed
    # matmuls write every psum partition they cover.
    bandt = np.zeros((TAIL_IN, KW, TAIL_M), dtype=np.float32)
    ot_ = np.arange(TAIL_OUT)
    for di in range(KH):
        for dj in range(KW):
            bandt[ot_ + di, dj, ot_] = weight[di, dj]
    bandt = bandt.astype(NP_BF16)

    starts = [min(i * ROWS_PER_CORE, H - SLAB_IN_ROWS) for i in range(N_CORES)]
    in_maps = [
        {
            "xs": np.ascontiguousarray(xbf[s0 : s0 + SLAB_IN_ROWS]),
            "bands": bands,
            "bandt": bandt,
        }
        for s0 in starts
    ]

    # The shared device occasionally returns corrupted results after an NRT
    # wedge (observed once across dozens of runs: rel err jumped ~12 orders of
    # magnitude on an unchanged binary). A handful of sampled rows checked
    # against a host conv (~2M flops) catches that reliably — the legit bf16
    # error is ~1e-2 absolute while corruption shows up as O(10+) — so retry
    # the device run when the spot check fails.
    for attempt in range(3):
        res = run_bass_kernel_spmd(nc, in_maps, core_ids=list(range(N_CORES)))
        LAST_RESULTS = res

        full = np.empty((OH, OW), dtype=np.float32)
        for i in range(N_CORES - 1):
            full[i * ROWS_PER_CORE : (i + 1) * ROWS_PER_CORE] = np.asarray(
                res.results[i]["out"], dtype=np.float32
            )
        # Core 7's slab starts at row 7166, so its first 2 output rows
        # duplicate core 6's last 2; keep rows 2.. (= conv rows 7168..8189).
        full[(N_CORES - 1) * ROWS_PER_CORE :] = np.asarray(
            res.results[N_CORES - 1]["out"], dtype=np.float32
        )[ROWS_PER_CORE - (OH - (N_CORES - 1) * ROWS_PER_CORE) :]
        if bias[0] != 0.0:
            full += bias[0]
        if _spot_check(full, xbf, weight, bias[0]):
            return full
        print(
            f"kernel: device output failed spot check (attempt {attempt + 1}); "
            "retrying",
            file=sys.stderr,
        )
    return full


def _spot_check(full, xbf, w, bias):
    rows = set()
    for i in range(N_CORES):
        base = i * ROWS_PER_CORE
        rows.update((base, base + 513, base + SLAB_OUT * 4, base + 1023))
    rows.add(OH - 1)
    wq = w.astype(NP_BF16).astype(np.float32)
    for r in sorted(rows):
        if r >= OH:
            continue
        xr = xbf[r : r + KH].astype(np.float32)
        ref = np.zeros(OW, dtype=np.float32)
        for di in range(KH):
            for dj in range(KW):
                ref += wq[di, dj] * xr[di, dj : dj + OW]
        ref += bias
        tol = max(0.05 * float(np.abs(ref).max()), 0.05)
        if float(np.abs(full[r] - ref).max()) > tol:
            return False
    return True
